# revision 10
# baseline (speedup 1.0000x reference)
"""CBConv2d (change-based conv) Trainium2 kernel, 8-core SPMD.

Reference semantics (B=1, C=64, H=W=512, 3x3 SAME conv):
  changed = any_c(|inp - prev_input| > 0.1)            # [H, W]
  dilated = maxpool3x3(changed)                        # [H, W]
  out     = dilated ? (conv2d(inp, w) + bias) : prev_output

Sharding: H split across 8 cores (64 rows each), halos materialized on host.

Per-core device pipeline (4 tiles of 16 output rows):
  - inputs bf16 (host pre-cast); prev_output and out are ALSO bf16 on the
    wire (upcast to fp32 on host) -- tolerance is 2e-2, bf16 adds ~4e-3.
  - conv runs on TensorE in 64x64 array-tiled mode: 4 concurrent K=64
    matmuls in the 4 array quadrants (T0/T2/T8/T10), one output row each,
    rows paired (s, s+4) within each 8-row half -> PSUM banks hold
    [row j | row j+4] across the partition halves. Measured 80.6 ns/MM vs
    325 ns/MM for the 128-contraction block-diagonal scheme.
  - change mask: DVE subtract, ACT Square, DVE is_gt -> 0/1 indicator;
    change count + H-dilation from banded-ones matmuls split into two
    concurrent 64-contraction quadrant MMs (cntA from group0, cntB from
    group1, summed by DVE into the W-dilation buffer); W-dilation is 2
    DVE adds; col-split PE ones-matmuls broadcast the dilated count
    across partitions into PSUM; one copy_predicated per pair-block
    merges conv over prev_output.
  - software-pipelined emission: copy_predicated for blocks 4..7 of tile
    t runs at the start of tile t+1 (with the out-DMA), so DVE has ready
    work while tile t+1's inputs land and PE never stalls on the merge.

Mask exactness note: inputs are bf16-rounded, so pixels whose |diff| sits
within ~0.4% of the threshold can flip vs the fp32 reference. A flipped
pixel only affects the output if its entire 3x3 neighborhood has no other
changed pixel; with this data distribution (~95% changed) the expected
number of affected output pixels is ~1e-7.
"""
import numpy as np
import ml_dtypes

import concourse.bass as bass
import concourse.mybir as mybir
import concourse.tile as tile
from concourse import bacc
from concourse.bass_utils import run_bass_kernel_spmd

F32 = mybir.dt.float32
BF16 = mybir.dt.bfloat16
BF = ml_dtypes.bfloat16

C = 64          # channels
H = W = 512     # spatial
NCORES = 8
RPC = H // NCORES          # rows per core (64)
R = 16                     # output rows per tile
NT = RPC // R              # tiles per core (4)
NPAD = R + 2               # padded rows per tile (18)
G = 10                     # rows per partition-group (overlapping: lower=0..9, upper=8..17)
WP = W + 2                 # padded width (514)
THR = float(np.float32(0.1))

# pair-block structure: block b of the [128, 8*W] pout/out tiles holds
# out row LROW[b] on partitions 0:64 and UROW[b] on partitions 64:128.
LROW = [0, 1, 2, 3, 8, 9, 10, 11]
UROW = [4, 5, 6, 7, 12, 13, 14, 15]
# cnt/dil row u corresponds to out row ROWPERM[u] (so dil1 = natural reshape)
ROWPERM = LROW + UROW

_cached = {}


def build_nc(loop_iters: int = 0, variant: str = "full"):
    """Build the per-core Bass program. loop_iters>0 wraps the whole pipeline
    in a For_i loop that re-executes it (for slope-based timing).

    variant tokens (comma-joined):
      indact  - indicator via ACT Square + Relu(bias) instead of DVE is_gt
      nosel   - plain copy instead of copy_predicated
      nomb    - also skip mask-broadcast matmuls
      nodil   - also skip W-dilation + dil1 DMA
      nocnt   - also skip count matmuls
      noind   - also skip indicator ops (pure conv kernel)
      noconv  - skip conv matmuls + evac (mask pipeline only; copy prev->out)
    """
    has_ind = "noind" not in variant
    has_cnt = has_ind and "nocnt" not in variant
    has_dil = has_cnt and "nodil" not in variant
    has_mb = has_dil and "nomb" not in variant
    has_sel = has_mb and "nosel" not in variant
    has_conv = "noconv" not in variant
    ind_act = "indact" in variant

    nc = bacc.Bacc("TRN2", target_bir_lowering=False, debug=False,
                   enable_asserts=True, num_devices=NCORES)

    xin = nc.dram_tensor("xin", [NT, 128, G * WP], BF16, kind="ExternalInput")
    pin = nc.dram_tensor("pin", [NT, 128, G * WP], BF16, kind="ExternalInput")
    pout = nc.dram_tensor("pout", [NT, 128, 8 * W], BF16, kind="ExternalInput")
    wt = nc.dram_tensor("wt", [128, 9 * 64], BF16, kind="ExternalInput")
    sel = nc.dram_tensor("sel", [128, G * R], BF16, kind="ExternalInput")
    sel2x = nc.dram_tensor("sel2x", [2, 128], BF16, kind="ExternalInput")
    biasv = nc.dram_tensor("biasv", [128, 1], F32, kind="ExternalInput")
    outd = nc.dram_tensor("out", [NT, 128, 8 * W], BF16, kind="ExternalOutput")

    with tile.TileContext(nc) as tc:
        with tc.tile_pool(name="consts", bufs=1) as cpool, \
             tc.tile_pool(name="io", bufs=2) as iopool, \
             tc.tile_pool(name="mask", bufs=2) as mpool, \
             tc.tile_pool(name="cnt", bufs=1, space="PSUM") as cntpool, \
             tc.tile_pool(name="conv", bufs=2, space="PSUM") as convpool, \
             tc.tile_pool(name="mb", bufs=3, space="PSUM") as mbpool:

            wtt = cpool.tile([128, 9 * 64], BF16)
            selt = cpool.tile([128, G * R], BF16)
            sel2xt = cpool.tile([2, 128], BF16)
            biast = cpool.tile([128, 1], F32)
            negthr2 = cpool.tile([128, 1], F32)
            hs = cpool.tile([R, WP], F32)       # persistent zero-padded edges
            nc.sync.dma_start(out=wtt[:], in_=wt[:])
            nc.sync.dma_start(out=selt[:], in_=sel[:])
            nc.sync.dma_start(out=sel2xt[:], in_=sel2x[:])
            nc.sync.dma_start(out=biast[:], in_=biasv[:])
            nc.vector.memset(negthr2[:], -(THR * THR))
            nc.vector.memset(hs[:], 0.0)

            taps = [(dh, dw) for dh in range(3) for dw in range(3)]

            def emit_merge(prev, blocks):
                """copy_predicated (or copy) conv over prev_output, then
                DMA the finished tile out."""
                if prev is None:
                    return
                pvt, conv_sb, mbs, t = prev
                for b in blocks:
                    sl = slice(b * W, (b + 1) * W)
                    if has_sel and has_conv:
                        nc.vector.copy_predicated(
                            pvt[:, sl], mbs[b][:].bitcast(mybir.dt.int32),
                            conv_sb[:, sl])
                    elif has_conv:
                        nc.vector.tensor_copy(out=pvt[:, sl],
                                              in_=conv_sb[:, sl])
                if blocks[-1] == 7:
                    nc.scalar.dma_start(out=outd[t], in_=pvt[:])

            def conv_slot(xt, conv_sb, s):
                """One pair-slot: 4 quadrant MM chains for rows
                (s, s+4, 8+s, 12+s), then evacuate with bias."""
                cbA = convpool.tile([128, W], F32, tag="cbA", name="cbA")
                cbB = convpool.tile([128, W], F32, tag="cbB", name="cbB")
                for i, (dh, dw) in enumerate(taps):
                    ti = dh * 3 + dw
                    st, sp = (i == 0), (i == len(taps) - 1)
                    wlo = wtt[0:64, ti * 64:(ti + 1) * 64]
                    whi = wtt[64:128, ti * 64:(ti + 1) * 64]
                    nc.tensor.matmul(
                        cbA[0:64], wlo,
                        xt[0:64, (s + dh) * WP + dw:(s + dh) * WP + dw + W],
                        start=st, stop=sp)
                    nc.tensor.matmul(
                        cbA[64:128], wlo,
                        xt[0:64, (s + 4 + dh) * WP + dw:
                           (s + 4 + dh) * WP + dw + W],
                        start=st, stop=sp)
                    nc.tensor.matmul(
                        cbB[0:64], whi,
                        xt[64:128, (s + dh) * WP + dw:
                           (s + dh) * WP + dw + W],
                        start=st, stop=sp)
                    nc.tensor.matmul(
                        cbB[64:128], whi,
                        xt[64:128, (s + 4 + dh) * WP + dw:
                           (s + 4 + dh) * WP + dw + W],
                        start=st, stop=sp)
                for cb, b in ((cbA, s), (cbB, 4 + s)):
                    nc.scalar.activation(
                        conv_sb[:, b * W:(b + 1) * W], cb[:],
                        mybir.ActivationFunctionType.Identity,
                        bias=biast[:])

            def emit_tile(t, prev):
                # --- loads (sync queue) ---
                xt = iopool.tile([128, G * WP], BF16, tag="xt")
                pt = iopool.tile([128, G * WP], BF16, tag="pt")
                pvt = iopool.tile([128, 8 * W], BF16, tag="pvt")
                nc.sync.dma_start(out=xt[:], in_=xin[t])
                nc.sync.dma_start(out=pt[:], in_=pin[t])
                nc.sync.dma_start(out=pvt[:], in_=pout[t])

                # --- lagged merge for previous tile (ready work for DVE) ---
                emit_merge(prev, [4, 5, 6, 7])

                # --- change indicator ---
                if has_ind:
                    ind = mpool.tile([128, G * WP], BF16, tag="ind")
                    nc.vector.tensor_tensor(out=ind[:], in0=xt[:], in1=pt[:],
                                            op=mybir.AluOpType.subtract)
                    nc.scalar.activation(ind[:], ind[:],
                                         mybir.ActivationFunctionType.Square)
                    if ind_act:
                        nc.scalar.activation(ind[:], ind[:],
                                             mybir.ActivationFunctionType.Relu,
                                             bias=negthr2[:])
                    else:
                        nc.vector.tensor_scalar(out=ind[:], in0=ind[:],
                                                scalar1=THR * THR,
                                                scalar2=None,
                                                op0=mybir.AluOpType.is_gt)

                conv_sb = iopool.tile([128, 8 * W], BF16, tag="conv_sb")
                if has_conv:
                    for s in (0, 1, 2):
                        conv_slot(xt, conv_sb, s)

                if has_cnt:
                    # --- change count + H-dilation via banded matmuls
                    # (cnt row u = out row ROWPERM[u]) ---
                    cnt = cntpool.tile([R, W], F32, tag="cnt")
                    for k in range(G):
                        nc.tensor.matmul(
                            cnt[:], selt[:, k * R:(k + 1) * R],
                            ind[:, k * WP + 1:k * WP + 1 + W],
                            start=(k == 0), stop=(k == G - 1))

                dil1 = None
                if has_dil:
                    # --- W-dilation on [R, W+2] (hs edges stay zero) ---
                    nc.vector.tensor_copy(out=hs[:, 1:W + 1], in_=cnt[:])
                    t1 = mpool.tile([R, W + 1], F32, tag="t1")
                    nc.vector.tensor_tensor(out=t1[:], in0=hs[:, 0:W + 1],
                                            in1=hs[:, 1:WP],
                                            op=mybir.AluOpType.add)
                    dil = mpool.tile([R, W], BF16, tag="dil")
                    nc.vector.tensor_tensor(out=dil[:], in0=t1[:, 0:W],
                                            in1=hs[:, 2:WP],
                                            op=mybir.AluOpType.add)
                    dil1 = mpool.tile([2, 8 * W], BF16, tag="dil1")
                    nc.scalar.dma_start(out=dil1[:], in_=dil[:])

                if has_conv:
                    conv_slot(xt, conv_sb, 3)

                mbs = []
                if has_mb:
                    # --- broadcast dilated counts: col-split pair MMs ---
                    for b in range(8):
                        mb = mbpool.tile([128, W], F32, tag="mb")
                        nc.tensor.matmul(mb[0:64], sel2xt[0:2, 0:64],
                                         dil1[:, b * W:(b + 1) * W],
                                         start=True, stop=True)
                        nc.tensor.matmul(mb[64:128], sel2xt[0:2, 64:128],
                                         dil1[:, b * W:(b + 1) * W],
                                         start=True, stop=True)
                        mbs.append(mb)

                cur = (pvt, conv_sb, mbs, t)
                emit_merge(cur, [0, 1, 2, 3])
                return cur

            def emit_all():
                prev = None
                for t in range(NT):
                    prev = emit_tile(t, prev)
                emit_merge(prev, [4, 5, 6, 7])

            if loop_iters > 0:
                with tc.For_i(0, loop_iters, 1,
                              hint_engines=(mybir.EngineType.PE,
                                            mybir.EngineType.DVE,
                                            mybir.EngineType.Activation,
                                            mybir.EngineType.SP)):
                    emit_all()
            else:
                emit_all()

    nc.compile()
    return nc


def host_prep(inp, prev_input, prev_output, weight, bias):
    """Build per-core in_maps."""
    inp = np.asarray(inp)
    prev_input = np.asarray(prev_input)
    prev_output = np.asarray(prev_output)
    weight = np.asarray(weight)
    bias = np.asarray(bias)

    xpad = np.zeros((C, H + 2, WP), dtype=BF)
    ppad = np.zeros((C, H + 2, WP), dtype=BF)
    xpad[:, 1:H + 1, 1:W + 1] = inp[0].astype(BF)
    ppad[:, 1:H + 1, 1:W + 1] = prev_input[0].astype(BF)

    # weights: wt[ci + 64g, (dh*3+dw)*64 + co] = weight[co, ci, dh, dw]
    wtap = weight.transpose(1, 2, 3, 0).reshape(C, 9 * C).astype(BF)
    wt = np.concatenate([wtap, wtap], axis=0)  # [128, 576]

    # sel bands, cnt rows permuted: cnt row u <-> out row ROWPERM[u].
    # group0 handles padded rows p=0..9 (covers out rows 0..7), group1
    # p=8..17 (out rows 8..15): sel[*, k*R + u] = 1 iff p-2 <= ROWPERM[u] <= p.
    selA = np.zeros((G, R), dtype=BF)
    selB = np.zeros((G, R), dtype=BF)
    for u in range(R):
        rr = ROWPERM[u]
        for p in range(rr, rr + 3):        # padded rows rr..rr+2
            if rr <= 7:
                selA[p, u] = 1
            else:
                selB[p - 8, u] = 1
    sel = np.empty((128, G * R), dtype=BF)
    sel[:64] = selA.reshape(1, G * R)
    sel[64:] = selB.reshape(1, G * R)

    sel2x = np.zeros((2, 128), dtype=BF)
    sel2x[0, :64] = 1
    sel2x[1, 64:] = 1
    biasv = np.tile(bias.astype(np.float32).reshape(-1, 1), (2, 1))  # [128,1]

    lrow = np.array(LROW)
    urow = np.array(UROW)

    in_maps = []
    for c in range(NCORES):
        r0 = c * RPC

        def slab(pad):
            s = np.empty((NT, 128, G * WP), dtype=BF)
            for t in range(NT):
                rows = pad[:, r0 + 16 * t: r0 + 16 * t + NPAD, :]  # [C,18,WP]
                s[t, :64] = rows[:, 0:10].reshape(C, G * WP)
                s[t, 64:] = rows[:, 8:18].reshape(C, G * WP)
            return s

        po = prev_output[0][:, r0:r0 + RPC, :].astype(BF)  # [C, 64, W]
        po = po.reshape(C, NT, R, W)
        pot = np.empty((NT, 128, 8 * W), dtype=BF)
        for t in range(NT):
            pot[t, :64] = po[:, t, lrow].reshape(C, 8 * W)
            pot[t, 64:] = po[:, t, urow].reshape(C, 8 * W)

        in_maps.append({
            "xin": slab(xpad), "pin": slab(ppad), "pout": pot,
            "wt": wt, "sel": sel, "sel2x": sel2x, "biasv": biasv,
        })
    return in_maps


def host_post(results):
    """Reassemble [NCORES] x [NT, 128, 8*W] bf16 -> [1, C, H, W] fp32."""
    out = np.empty((1, C, H, W), dtype=np.float32)
    lrow = np.array(LROW)
    urow = np.array(UROW)
    for c, res in enumerate(results):
        o = res["out"].reshape(NT, 2, C, 8, W).astype(np.float32)
        blk = np.empty((NT, C, R, W), dtype=np.float32)
        blk[:, :, lrow] = o[:, 0]
        blk[:, :, urow] = o[:, 1]
        out[0, :, c * RPC:(c + 1) * RPC, :] = \
            blk.transpose(1, 0, 2, 3).reshape(C, RPC, W)
    return out


def kernel(inp, prev_input, prev_output, weight, bias):
    if "nc" not in _cached:
        _cached["nc"] = build_nc(0)
    nc = _cached["nc"]
    in_maps = host_prep(inp, prev_input, prev_output, weight, bias)
    res = run_bass_kernel_spmd(nc, in_maps, core_ids=list(range(NCORES)))
    return host_post(res.results)


if __name__ == "__main__":
    rng = np.random.default_rng(0)
    inp = rng.standard_normal((1, C, H, W), dtype=np.float32)
    prev_input = inp + 0.05 * rng.standard_normal((1, C, H, W), dtype=np.float32)
    prev_output = rng.standard_normal((1, C, H, W), dtype=np.float32)
    weight = (0.05 * rng.standard_normal((C, C, 3, 3))).astype(np.float32)
    bias = rng.standard_normal(C).astype(np.float32)
    out = kernel(inp=inp, prev_input=prev_input, prev_output=prev_output,
                 weight=weight, bias=bias)
    print("out", out.shape, out.dtype, float(np.abs(out).mean()))


# revision 13
# speedup vs baseline: 1.0038x; 1.0038x over previous
"""CBConv2d (change-based conv) Trainium2 kernel, 8-core SPMD.

Reference semantics (B=1, C=64, H=W=512, 3x3 SAME conv):
  changed = any_c(|inp - prev_input| > 0.1)            # [H, W]
  dilated = maxpool3x3(changed)                        # [H, W]
  out     = dilated ? (conv2d(inp, w) + bias) : prev_output

Sharding: H split across 8 cores (64 rows each), halos materialized on host.

Per-core device pipeline (4 tiles of 16 output rows):
  - inputs bf16 (host pre-cast); prev_output and out are ALSO bf16 on the
    wire (upcast to fp32 on host) -- tolerance is 2e-2, bf16 adds ~4e-3.
  - conv runs on TensorE in 64x64 array-tiled mode: 4 concurrent K=64
    matmuls in the 4 array quadrants (T0/T2/T8/T10), one output row each,
    rows paired (s, s+4) within each 8-row half -> PSUM banks hold
    [row j | row j+4] across the partition halves. Measured 80.6 ns/MM vs
    325 ns/MM for the 128-contraction block-diagonal scheme.
  - change mask: DVE subtract, ACT Square, DVE is_gt -> 0/1 indicator;
    change count + H-dilation from banded-ones matmuls split into two
    concurrent 64-contraction quadrant MMs (cntA from group0, cntB from
    group1, summed by DVE into the W-dilation buffer); W-dilation is 2
    DVE adds; col-split PE ones-matmuls broadcast the dilated count
    across partitions into PSUM; one copy_predicated per pair-block
    merges conv over prev_output.
  - software-pipelined emission: copy_predicated for blocks 4..7 of tile
    t runs at the start of tile t+1 (with the out-DMA), so DVE has ready
    work while tile t+1's inputs land and PE never stalls on the merge.

Mask exactness note: inputs are bf16-rounded, so pixels whose |diff| sits
within ~0.4% of the threshold can flip vs the fp32 reference. A flipped
pixel only affects the output if its entire 3x3 neighborhood has no other
changed pixel; with this data distribution (~95% changed) the expected
number of affected output pixels is ~1e-7.
"""
import numpy as np
import ml_dtypes

import concourse.bass as bass
import concourse.mybir as mybir
import concourse.tile as tile
from concourse import bacc
from concourse.bass_utils import run_bass_kernel_spmd

F32 = mybir.dt.float32
BF16 = mybir.dt.bfloat16
BF = ml_dtypes.bfloat16

C = 64          # channels
H = W = 512     # spatial
NCORES = 8
RPC = H // NCORES          # rows per core (64)
R = 16                     # output rows per tile
NT = RPC // R              # tiles per core (4)
NPAD = R + 2               # padded rows per tile (18)
G = 10                     # rows per partition-group (overlapping: lower=0..9, upper=8..17)
WP = W + 2                 # padded width (514)
THR = float(np.float32(0.1))

# pair-block structure: block b of the [128, 8*W] pout/out tiles holds
# out row LROW[b] on partitions 0:64 and UROW[b] on partitions 64:128.
LROW = [0, 1, 2, 3, 8, 9, 10, 11]
UROW = [4, 5, 6, 7, 12, 13, 14, 15]
# cnt/dil row u corresponds to out row ROWPERM[u] (so dil1 = natural reshape)
ROWPERM = LROW + UROW

_cached = {}


def build_nc(loop_iters: int = 0, variant: str = "full"):
    """Build the per-core Bass program. loop_iters>0 wraps the whole pipeline
    in a For_i loop that re-executes it (for slope-based timing).

    variant tokens (comma-joined):
      indact  - indicator via ACT Square + Relu(bias) instead of DVE is_gt
      nosel   - plain copy instead of copy_predicated
      nomb    - also skip mask-broadcast matmuls
      nodil   - also skip W-dilation + dil1 DMA
      nocnt   - also skip count matmuls
      noind   - also skip indicator ops (pure conv kernel)
      noconv  - skip conv matmuls + evac (mask pipeline only; copy prev->out)
    """
    has_ind = "noind" not in variant
    has_cnt = has_ind and "nocnt" not in variant
    has_dil = has_cnt and "nodil" not in variant
    has_mb = has_dil and "nomb" not in variant
    has_sel = has_mb and "nosel" not in variant
    has_conv = "noconv" not in variant
    ind_act = "indact" in variant

    nc = bacc.Bacc("TRN2", target_bir_lowering=False, debug=False,
                   enable_asserts=True, num_devices=NCORES)

    xin = nc.dram_tensor("xin", [NT, 128, G * WP], BF16, kind="ExternalInput")
    pin = nc.dram_tensor("pin", [NT, 128, G * WP], BF16, kind="ExternalInput")
    pout = nc.dram_tensor("pout", [NT, 128, 8 * W], BF16, kind="ExternalInput")
    wt = nc.dram_tensor("wt", [128, 9 * 64], BF16, kind="ExternalInput")
    sel = nc.dram_tensor("sel", [128, G * R], BF16, kind="ExternalInput")
    sel2x = nc.dram_tensor("sel2x", [2, 128], BF16, kind="ExternalInput")
    biasv = nc.dram_tensor("biasv", [128, 1], F32, kind="ExternalInput")
    outd = nc.dram_tensor("out", [NT, 128, 8 * W], BF16, kind="ExternalOutput")

    with tile.TileContext(nc) as tc:
        with tc.tile_pool(name="consts", bufs=1) as cpool, \
             tc.tile_pool(name="io", bufs=2) as iopool, \
             tc.tile_pool(name="mask", bufs=2) as mpool, \
             tc.tile_pool(name="cnt", bufs=1, space="PSUM") as cntpool, \
             tc.tile_pool(name="conv", bufs=2, space="PSUM") as convpool, \
             tc.tile_pool(name="mb", bufs=3, space="PSUM") as mbpool:

            wtt = cpool.tile([128, 9 * 64], BF16)
            selt = cpool.tile([128, G * R], BF16)
            sel2xt = cpool.tile([2, 128], BF16)
            biast = cpool.tile([128, 1], F32)
            negthr2 = cpool.tile([128, 1], F32)
            hs = cpool.tile([R, WP], F32)       # persistent zero-padded edges
            nc.sync.dma_start(out=wtt[:], in_=wt[:])
            nc.sync.dma_start(out=selt[:], in_=sel[:])
            nc.sync.dma_start(out=sel2xt[:], in_=sel2x[:])
            nc.sync.dma_start(out=biast[:], in_=biasv[:])
            nc.vector.memset(negthr2[:], -(THR * THR))
            nc.vector.memset(hs[:], 0.0)

            taps = [(dh, dw) for dh in range(3) for dw in range(3)]

            def emit_merge(prev, blocks):
                """copy_predicated (or copy) conv over prev_output, then
                DMA the finished tile out."""
                if prev is None:
                    return
                pvt, conv_sb, mbs, t = prev
                for b in blocks:
                    sl = slice(b * W, (b + 1) * W)
                    pp = 2 * b if b < 4 else 2 * (b - 4) + 1
                    csl = slice(pp * W, (pp + 1) * W)
                    if has_sel and has_conv:
                        nc.vector.copy_predicated(
                            pvt[:, sl], mbs[b][:].bitcast(mybir.dt.int32),
                            conv_sb[:, csl])
                    elif has_conv:
                        nc.vector.tensor_copy(out=pvt[:, sl],
                                              in_=conv_sb[:, csl])
                if blocks[-1] == 7:
                    nc.scalar.dma_start(out=outd[t], in_=pvt[:])

            def conv_slot(xt, conv_sb, s):
                """One pair-slot: 4 quadrant MM chains for rows
                (s, s+4, 8+s, 12+s) into one 2-bank PSUM tile (block s in
                cols 0:W from T0/T2, block 4+s in W:2W from T8/T10), then
                a single paired evacuation with bias. conv_sb stores
                pair-interleaved: block b at column pairpos(b)*W."""
                cb = convpool.tile([128, 2 * W], F32, tag="cb", name="cb")
                for i, (dh, dw) in enumerate(taps):
                    ti = dh * 3 + dw
                    st, sp = (i == 0), (i == len(taps) - 1)
                    wlo = wtt[0:64, ti * 64:(ti + 1) * 64]
                    whi = wtt[64:128, ti * 64:(ti + 1) * 64]
                    nc.tensor.matmul(
                        cb[0:64, 0:W], wlo,
                        xt[0:64, (s + dh) * WP + dw:(s + dh) * WP + dw + W],
                        start=st, stop=sp)
                    nc.tensor.matmul(
                        cb[64:128, 0:W], wlo,
                        xt[0:64, (s + 4 + dh) * WP + dw:
                           (s + 4 + dh) * WP + dw + W],
                        start=st, stop=sp)
                    nc.tensor.matmul(
                        cb[0:64, W:2 * W], whi,
                        xt[64:128, (s + dh) * WP + dw:
                           (s + dh) * WP + dw + W],
                        start=st, stop=sp)
                    nc.tensor.matmul(
                        cb[64:128, W:2 * W], whi,
                        xt[64:128, (s + 4 + dh) * WP + dw:
                           (s + 4 + dh) * WP + dw + W],
                        start=st, stop=sp)
                nc.scalar.activation(
                    conv_sb[:, 2 * s * W:(2 * s + 2) * W], cb[:],
                    mybir.ActivationFunctionType.Identity,
                    bias=biast[:])

            def emit_tile(t, prev):
                # --- loads (sync queue) ---
                xt = iopool.tile([128, G * WP], BF16, tag="xt")
                pt = iopool.tile([128, G * WP], BF16, tag="pt")
                pvt = iopool.tile([128, 8 * W], BF16, tag="pvt")
                nc.sync.dma_start(out=xt[:], in_=xin[t])
                nc.sync.dma_start(out=pt[:], in_=pin[t])
                nc.sync.dma_start(out=pvt[:], in_=pout[t])

                # --- lagged merge for previous tile (ready work for DVE) ---
                emit_merge(prev, [4, 5, 6, 7])

                # --- change indicator ---
                if has_ind:
                    ind = mpool.tile([128, G * WP], BF16, tag="ind")
                    nc.vector.tensor_tensor(out=ind[:], in0=xt[:], in1=pt[:],
                                            op=mybir.AluOpType.subtract)
                    nc.scalar.activation(ind[:], ind[:],
                                         mybir.ActivationFunctionType.Square)
                    if ind_act:
                        nc.scalar.activation(ind[:], ind[:],
                                             mybir.ActivationFunctionType.Relu,
                                             bias=negthr2[:])
                    else:
                        nc.vector.tensor_scalar(out=ind[:], in0=ind[:],
                                                scalar1=THR * THR,
                                                scalar2=None,
                                                op0=mybir.AluOpType.is_gt)

                conv_sb = iopool.tile([128, 8 * W], BF16, tag="conv_sb")
                if has_conv:
                    for s in (0, 1, 2):
                        conv_slot(xt, conv_sb, s)

                if has_cnt:
                    # --- change count + H-dilation via banded matmuls
                    # (cnt row u = out row ROWPERM[u]) ---
                    cnt = cntpool.tile([R, W], F32, tag="cnt")
                    for k in range(G):
                        nc.tensor.matmul(
                            cnt[:], selt[:, k * R:(k + 1) * R],
                            ind[:, k * WP + 1:k * WP + 1 + W],
                            start=(k == 0), stop=(k == G - 1))

                dil1 = None
                if has_dil:
                    # --- W-dilation on [R, W+2] (hs edges stay zero) ---
                    nc.vector.tensor_copy(out=hs[:, 1:W + 1], in_=cnt[:])
                    t1 = mpool.tile([R, W + 1], F32, tag="t1")
                    nc.vector.tensor_tensor(out=t1[:], in0=hs[:, 0:W + 1],
                                            in1=hs[:, 1:WP],
                                            op=mybir.AluOpType.add)
                    dil = mpool.tile([R, W], BF16, tag="dil")
                    nc.vector.tensor_tensor(out=dil[:], in0=t1[:, 0:W],
                                            in1=hs[:, 2:WP],
                                            op=mybir.AluOpType.add)
                    dil1 = mpool.tile([2, 8 * W], BF16, tag="dil1")
                    nc.scalar.dma_start(out=dil1[:], in_=dil[:])

                if has_conv:
                    conv_slot(xt, conv_sb, 3)

                mbs = []
                if has_mb:
                    # --- broadcast dilated counts across partitions ---
                    for b in range(8):
                        mb = mbpool.tile([128, W], F32, tag="mb")
                        nc.tensor.matmul(mb[:], sel2xt[:],
                                         dil1[:, b * W:(b + 1) * W],
                                         start=True, stop=True)
                        mbs.append(mb)

                cur = (pvt, conv_sb, mbs, t)
                emit_merge(cur, [0, 1, 2, 3])
                return cur

            def emit_all():
                prev = None
                for t in range(NT):
                    prev = emit_tile(t, prev)
                emit_merge(prev, [4, 5, 6, 7])

            if loop_iters > 0:
                with tc.For_i(0, loop_iters, 1,
                              hint_engines=(mybir.EngineType.PE,
                                            mybir.EngineType.DVE,
                                            mybir.EngineType.Activation,
                                            mybir.EngineType.SP)):
                    emit_all()
            else:
                emit_all()

    nc.compile()
    return nc


def host_prep(inp, prev_input, prev_output, weight, bias):
    """Build per-core in_maps."""
    inp = np.asarray(inp)
    prev_input = np.asarray(prev_input)
    prev_output = np.asarray(prev_output)
    weight = np.asarray(weight)
    bias = np.asarray(bias)

    xpad = np.zeros((C, H + 2, WP), dtype=BF)
    ppad = np.zeros((C, H + 2, WP), dtype=BF)
    xpad[:, 1:H + 1, 1:W + 1] = inp[0].astype(BF)
    ppad[:, 1:H + 1, 1:W + 1] = prev_input[0].astype(BF)

    # weights: wt[ci + 64g, (dh*3+dw)*64 + co] = weight[co, ci, dh, dw]
    wtap = weight.transpose(1, 2, 3, 0).reshape(C, 9 * C).astype(BF)
    wt = np.concatenate([wtap, wtap], axis=0)  # [128, 576]

    # sel bands, cnt rows permuted: cnt row u <-> out row ROWPERM[u].
    # group0 handles padded rows p=0..9 (covers out rows 0..7), group1
    # p=8..17 (out rows 8..15): sel[*, k*R + u] = 1 iff p-2 <= ROWPERM[u] <= p.
    selA = np.zeros((G, R), dtype=BF)
    selB = np.zeros((G, R), dtype=BF)
    for u in range(R):
        rr = ROWPERM[u]
        for p in range(rr, rr + 3):        # padded rows rr..rr+2
            if rr <= 7:
                selA[p, u] = 1
            else:
                selB[p - 8, u] = 1
    sel = np.empty((128, G * R), dtype=BF)
    sel[:64] = selA.reshape(1, G * R)
    sel[64:] = selB.reshape(1, G * R)

    sel2x = np.zeros((2, 128), dtype=BF)
    sel2x[0, :64] = 1
    sel2x[1, 64:] = 1
    biasv = np.tile(bias.astype(np.float32).reshape(-1, 1), (2, 1))  # [128,1]

    lrow = np.array(LROW)
    urow = np.array(UROW)

    in_maps = []
    for c in range(NCORES):
        r0 = c * RPC

        def slab(pad):
            s = np.empty((NT, 128, G * WP), dtype=BF)
            for t in range(NT):
                rows = pad[:, r0 + 16 * t: r0 + 16 * t + NPAD, :]  # [C,18,WP]
                s[t, :64] = rows[:, 0:10].reshape(C, G * WP)
                s[t, 64:] = rows[:, 8:18].reshape(C, G * WP)
            return s

        po = prev_output[0][:, r0:r0 + RPC, :].astype(BF)  # [C, 64, W]
        po = po.reshape(C, NT, R, W)
        pot = np.empty((NT, 128, 8 * W), dtype=BF)
        for t in range(NT):
            pot[t, :64] = po[:, t, lrow].reshape(C, 8 * W)
            pot[t, 64:] = po[:, t, urow].reshape(C, 8 * W)

        in_maps.append({
            "xin": slab(xpad), "pin": slab(ppad), "pout": pot,
            "wt": wt, "sel": sel, "sel2x": sel2x, "biasv": biasv,
        })
    return in_maps


def host_post(results):
    """Reassemble [NCORES] x [NT, 128, 8*W] bf16 -> [1, C, H, W] fp32."""
    out = np.empty((1, C, H, W), dtype=np.float32)
    lrow = np.array(LROW)
    urow = np.array(UROW)
    for c, res in enumerate(results):
        o = res["out"].reshape(NT, 2, C, 8, W).astype(np.float32)
        blk = np.empty((NT, C, R, W), dtype=np.float32)
        blk[:, :, lrow] = o[:, 0]
        blk[:, :, urow] = o[:, 1]
        out[0, :, c * RPC:(c + 1) * RPC, :] = \
            blk.transpose(1, 0, 2, 3).reshape(C, RPC, W)
    return out


def kernel(inp, prev_input, prev_output, weight, bias):
    if "nc" not in _cached:
        _cached["nc"] = build_nc(0)
    nc = _cached["nc"]
    in_maps = host_prep(inp, prev_input, prev_output, weight, bias)
    res = run_bass_kernel_spmd(nc, in_maps, core_ids=list(range(NCORES)))
    return host_post(res.results)


if __name__ == "__main__":
    rng = np.random.default_rng(0)
    inp = rng.standard_normal((1, C, H, W), dtype=np.float32)
    prev_input = inp + 0.05 * rng.standard_normal((1, C, H, W), dtype=np.float32)
    prev_output = rng.standard_normal((1, C, H, W), dtype=np.float32)
    weight = (0.05 * rng.standard_normal((C, C, 3, 3))).astype(np.float32)
    bias = rng.standard_normal(C).astype(np.float32)
    out = kernel(inp=inp, prev_input=prev_input, prev_output=prev_output,
                 weight=weight, bias=bias)
    print("out", out.shape, out.dtype, float(np.abs(out).mean()))


# revision 17
# speedup vs baseline: 1.0992x; 1.0950x over previous
"""CBConv2d (change-based conv) Trainium2 kernel, 8-core SPMD.

Reference semantics (B=1, C=64, H=W=512, 3x3 SAME conv):
  changed = any_c(|inp - prev_input| > 0.1)            # [H, W]
  dilated = maxpool3x3(changed)                        # [H, W]
  out     = dilated ? (conv2d(inp, w) + bias) : prev_output

Sharding: H split across 8 cores (64 rows each), halos materialized on host.

Per-core device pipeline (4 tiles of 16 output rows):
  - inputs bf16 (host pre-cast); prev_output and out are ALSO bf16 on the
    wire (upcast to fp32 on host) -- tolerance is 2e-2, bf16 adds ~4e-3.
  - conv runs on TensorE in 64x64 array-tiled mode: 4 concurrent K=64
    matmuls in the 4 array quadrants (T0/T2/T8/T10), one output row each,
    rows paired (s, s+4) within each 8-row half -> PSUM banks hold
    [row j | row j+4] across the partition halves. Measured 80.6 ns/MM vs
    325 ns/MM for the 128-contraction block-diagonal scheme.
  - change mask: DVE subtract, ACT Square, DVE is_gt -> 0/1 indicator;
    change count + H-dilation from banded-ones matmuls split into two
    concurrent 64-contraction quadrant MMs (cntA from group0, cntB from
    group1, summed by DVE into the W-dilation buffer); W-dilation is 2
    DVE adds; col-split PE ones-matmuls broadcast the dilated count
    across partitions into PSUM; one copy_predicated per pair-block
    merges conv over prev_output.
  - software-pipelined emission: copy_predicated for blocks 4..7 of tile
    t runs at the start of tile t+1 (with the out-DMA), so DVE has ready
    work while tile t+1's inputs land and PE never stalls on the merge.

Mask exactness note: inputs are bf16-rounded, so pixels whose |diff| sits
within ~0.4% of the threshold can flip vs the fp32 reference. A flipped
pixel only affects the output if its entire 3x3 neighborhood has no other
changed pixel; with this data distribution (~95% changed) the expected
number of affected output pixels is ~1e-7.
"""
import numpy as np
import ml_dtypes

import concourse.bass as bass
import concourse.mybir as mybir
import concourse.tile as tile
from concourse import bacc
from concourse.bass_utils import run_bass_kernel_spmd

F32 = mybir.dt.float32
BF16 = mybir.dt.bfloat16
BF = ml_dtypes.bfloat16

C = 64          # channels
H = W = 512     # spatial
NCORES = 8
RPC = H // NCORES          # rows per core (64)
R = 16                     # output rows per tile
NT = RPC // R              # tiles per core (4)
NPAD = R + 2               # padded rows per tile (18)
G = 10                     # rows per partition-group (overlapping: lower=0..9, upper=8..17)
WP = W + 2                 # padded width (514)
THR = float(np.float32(0.1))

# pair-block structure: block b of the [128, 8*W] pout/out tiles holds
# out row LROW[b] on partitions 0:64 and UROW[b] on partitions 64:128.
LROW = [0, 1, 2, 3, 8, 9, 10, 11]
UROW = [4, 5, 6, 7, 12, 13, 14, 15]
# cnt/dil row u corresponds to out row ROWPERM[u] (so dil1 = natural reshape)
ROWPERM = LROW + UROW

_cached = {}


def build_nc(loop_iters: int = 0, variant: str = "full"):
    """Build the per-core Bass program. loop_iters>0 wraps the whole pipeline
    in a For_i loop that re-executes it (for slope-based timing).

    variant tokens (comma-joined):
      indact  - indicator via ACT Square + Relu(bias) instead of DVE is_gt
      nosel   - plain copy instead of copy_predicated
      nomb    - also skip mask-broadcast matmuls
      nodil   - also skip W-dilation + dil1 DMA
      nocnt   - also skip count matmuls
      noind   - also skip indicator ops (pure conv kernel)
      noconv  - skip conv matmuls + evac (mask pipeline only; copy prev->out)
    """
    has_ind = "noind" not in variant
    has_cnt = has_ind and "nocnt" not in variant
    has_dil = has_cnt and "nodil" not in variant
    has_mb = has_dil and "nomb" not in variant
    has_sel = has_mb and "nosel" not in variant
    has_conv = "noconv" not in variant
    ind_act = "indact" in variant

    nc = bacc.Bacc("TRN2", target_bir_lowering=False, debug=False,
                   enable_asserts=True, num_devices=NCORES)

    xin = nc.dram_tensor("xin", [NT, 128, G * WP], BF16, kind="ExternalInput")
    pin = nc.dram_tensor("pin", [NT, 128, G * WP], BF16, kind="ExternalInput")
    pout = nc.dram_tensor("pout", [NT, 128, 8 * W], BF16, kind="ExternalInput")
    wt = nc.dram_tensor("wt", [128, 9 * 64], BF16, kind="ExternalInput")
    sel = nc.dram_tensor("sel", [128, G * R], BF16, kind="ExternalInput")
    sel2x = nc.dram_tensor("sel2x", [2, 128], BF16, kind="ExternalInput")
    biasv = nc.dram_tensor("biasv", [128, 1], F32, kind="ExternalInput")
    outd = nc.dram_tensor("out", [NT, 128, 8 * W], BF16, kind="ExternalOutput")

    with tile.TileContext(nc) as tc:
        with tc.tile_pool(name="consts", bufs=1) as cpool, \
             tc.tile_pool(name="io", bufs=2) as iopool, \
             tc.tile_pool(name="io3", bufs=3) as io3pool, \
             tc.tile_pool(name="mask", bufs=2) as mpool, \
             tc.tile_pool(name="cnt", bufs=1, space="PSUM") as cntpool, \
             tc.tile_pool(name="conv", bufs=2, space="PSUM") as convpool, \
             tc.tile_pool(name="mb", bufs=3, space="PSUM") as mbpool:

            wtt = cpool.tile([128, 9 * 64], BF16)
            selt = cpool.tile([128, G * R], BF16)
            sel2xt = cpool.tile([2, 128], BF16)
            biast = cpool.tile([128, 1], F32)
            negthr2 = cpool.tile([128, 1], F32)
            hs = cpool.tile([R, WP], F32)       # persistent zero-padded edges
            nc.sync.dma_start(out=wtt[:], in_=wt[:])
            nc.sync.dma_start(out=selt[:], in_=sel[:])
            nc.sync.dma_start(out=sel2xt[:], in_=sel2x[:])
            nc.sync.dma_start(out=biast[:], in_=biasv[:])
            nc.vector.memset(negthr2[:], -(THR * THR))
            nc.vector.memset(hs[:], 0.0)

            taps = [(dh, dw) for dh in range(3) for dw in range(3)]

            def emit_mb(st, blocks):
                """PE ones-matmuls broadcasting dilated counts for blocks."""
                if st is None or not has_mb:
                    return
                for b in blocks:
                    mb = mbpool.tile([128, W], F32, tag="mb")
                    nc.tensor.matmul(mb[:], sel2xt[:],
                                     st["dil1"][:, b * W:(b + 1) * W],
                                     start=True, stop=True)
                    st["mbs"][b] = mb

            def emit_cpred(st, blocks):
                """copy_predicated (or copy) conv over prev_output; after
                block 7, DMA the finished tile out."""
                if st is None:
                    return
                for b in blocks:
                    sl = slice(b * W, (b + 1) * W)
                    pp = 2 * b if b < 4 else 2 * (b - 4) + 1
                    csl = slice(pp * W, (pp + 1) * W)
                    if has_sel and has_conv:
                        nc.vector.copy_predicated(
                            st["pvt"][:, sl],
                            st["mbs"][b][:].bitcast(mybir.dt.int32),
                            st["conv_sb"][:, csl])
                    elif has_conv:
                        nc.vector.tensor_copy(out=st["pvt"][:, sl],
                                              in_=st["conv_sb"][:, csl])
                if blocks[-1] == 7:
                    nc.scalar.dma_start(out=outd[st["t"]], in_=st["pvt"][:])

            def emit_cnt_dil(st):
                """Count matmuls + W-dilation + dil1 gather for tile st."""
                if st is None or not has_cnt:
                    return
                cnt = cntpool.tile([R, W], F32, tag="cnt")
                for k in range(G):
                    nc.tensor.matmul(
                        cnt[:], selt[:, k * R:(k + 1) * R],
                        st["ind"][:, k * WP + 1:k * WP + 1 + W],
                        start=(k == 0), stop=(k == G - 1))
                if not has_dil:
                    return
                nc.vector.tensor_copy(out=hs[:, 1:W + 1], in_=cnt[:])
                t1 = mpool.tile([R, W + 1], F32, tag="t1")
                nc.vector.tensor_tensor(out=t1[:], in0=hs[:, 0:W + 1],
                                        in1=hs[:, 1:WP],
                                        op=mybir.AluOpType.add)
                dil = mpool.tile([R, W], BF16, tag="dil")
                nc.vector.tensor_tensor(out=dil[:], in0=t1[:, 0:W],
                                        in1=hs[:, 2:WP],
                                        op=mybir.AluOpType.add)
                dil1 = mpool.tile([2, 8 * W], BF16, tag="dil1")
                nc.scalar.dma_start(out=dil1[:], in_=dil[:])
                st["dil1"] = dil1

            def conv_slot(xt, conv_sb, s):
                """One pair-slot: 4 quadrant MM chains for rows
                (s, s+4, 8+s, 12+s) into one 2-bank PSUM tile (block s in
                cols 0:W from T0/T2, block 4+s in W:2W from T8/T10), then
                a single paired evacuation with bias. conv_sb stores
                pair-interleaved: block b at column pairpos(b)*W."""
                cb = convpool.tile([128, 2 * W], F32, tag="cb", name="cb")
                for i, (dh, dw) in enumerate(taps):
                    ti = dh * 3 + dw
                    st, sp = (i == 0), (i == len(taps) - 1)
                    wlo = wtt[0:64, ti * 64:(ti + 1) * 64]
                    whi = wtt[64:128, ti * 64:(ti + 1) * 64]
                    nc.tensor.matmul(
                        cb[0:64, 0:W], wlo,
                        xt[0:64, (s + dh) * WP + dw:(s + dh) * WP + dw + W],
                        start=st, stop=sp)
                    nc.tensor.matmul(
                        cb[64:128, 0:W], wlo,
                        xt[0:64, (s + 4 + dh) * WP + dw:
                           (s + 4 + dh) * WP + dw + W],
                        start=st, stop=sp)
                    nc.tensor.matmul(
                        cb[0:64, W:2 * W], whi,
                        xt[64:128, (s + dh) * WP + dw:
                           (s + dh) * WP + dw + W],
                        start=st, stop=sp)
                    nc.tensor.matmul(
                        cb[64:128, W:2 * W], whi,
                        xt[64:128, (s + 4 + dh) * WP + dw:
                           (s + 4 + dh) * WP + dw + W],
                        start=st, stop=sp)
                nc.scalar.activation(
                    conv_sb[:, 2 * s * W:(2 * s + 2) * W], cb[:],
                    mybir.ActivationFunctionType.Identity,
                    bias=biast[:])

            def emit_tile(t, p1, p2):
                """Lag-2 software pipeline: tile t emits its own loads,
                indicator, and conv; the count/dilation stage of tile t-1
                (p1); and the mask-broadcast + merge + store of tile t-2
                (p2), interleaved so no engine head-of-line blocks."""
                # --- loads (sync queue): xt first (conv), then pt (sub) ---
                xt = iopool.tile([128, G * WP], BF16, tag="xt")
                pt = iopool.tile([128, G * WP], BF16, tag="pt")
                pvt = io3pool.tile([128, 8 * W], BF16, tag="pvt")
                nc.sync.dma_start(out=xt[:], in_=xin[t])
                nc.sync.dma_start(out=pt[:], in_=pin[t])
                nc.sync.dma_start(out=pvt[:], in_=pout[t])
                st = {"t": t, "pvt": pvt, "mbs": [None] * 8}

                # --- change indicator (DVE sub -> ACT Square -> DVE gt) ---
                if has_ind:
                    ind = mpool.tile([128, G * WP], BF16, tag="ind")
                    st["ind"] = ind
                    nc.vector.tensor_tensor(out=ind[:], in0=xt[:], in1=pt[:],
                                            op=mybir.AluOpType.subtract)
                    nc.scalar.activation(ind[:], ind[:],
                                         mybir.ActivationFunctionType.Square)

                conv_sb = io3pool.tile([128, 8 * W], BF16, tag="conv_sb")
                st["conv_sb"] = conv_sb
                if has_conv:
                    conv_slot(xt, conv_sb, 0)
                    conv_slot(xt, conv_sb, 1)
                emit_cnt_dil(p1)
                emit_mb(p2, [0, 1, 2])
                if has_conv:
                    conv_slot(xt, conv_sb, 2)
                emit_cpred(p2, [0, 1])
                emit_mb(p2, [3, 4])
                emit_cpred(p2, [2, 3])
                if has_ind:
                    if ind_act:
                        nc.scalar.activation(ind[:], ind[:],
                                             mybir.ActivationFunctionType.Relu,
                                             bias=negthr2[:])
                    else:
                        nc.vector.tensor_scalar(out=ind[:], in0=ind[:],
                                                scalar1=THR * THR,
                                                scalar2=None,
                                                op0=mybir.AluOpType.is_gt)
                if has_conv:
                    conv_slot(xt, conv_sb, 3)
                emit_mb(p2, [5, 6])
                emit_cpred(p2, [4, 5])
                emit_mb(p2, [7])
                emit_cpred(p2, [6, 7])
                return st

            def drain_merge(st):
                emit_mb(st, [0, 1, 2])
                emit_cpred(st, [0, 1])
                emit_mb(st, [3, 4])
                emit_cpred(st, [2, 3])
                emit_mb(st, [5, 6])
                emit_cpred(st, [4, 5])
                emit_mb(st, [7])
                emit_cpred(st, [6, 7])

            def emit_all():
                p1 = p2 = None
                for t in range(NT):
                    p1, p2 = emit_tile(t, p1, p2), p1
                # drain: count/dil of tile 3, merges of tiles 2 and 3
                emit_cnt_dil(p1)
                drain_merge(p2)
                drain_merge(p1)

            if loop_iters > 0:
                with tc.For_i(0, loop_iters, 1,
                              hint_engines=(mybir.EngineType.PE,
                                            mybir.EngineType.DVE,
                                            mybir.EngineType.Activation,
                                            mybir.EngineType.SP)):
                    emit_all()
            else:
                emit_all()

    nc.compile()
    return nc


def host_prep(inp, prev_input, prev_output, weight, bias):
    """Build per-core in_maps."""
    inp = np.asarray(inp)
    prev_input = np.asarray(prev_input)
    prev_output = np.asarray(prev_output)
    weight = np.asarray(weight)
    bias = np.asarray(bias)

    xpad = np.zeros((C, H + 2, WP), dtype=BF)
    ppad = np.zeros((C, H + 2, WP), dtype=BF)
    xpad[:, 1:H + 1, 1:W + 1] = inp[0].astype(BF)
    ppad[:, 1:H + 1, 1:W + 1] = prev_input[0].astype(BF)

    # weights: wt[ci + 64g, (dh*3+dw)*64 + co] = weight[co, ci, dh, dw]
    wtap = weight.transpose(1, 2, 3, 0).reshape(C, 9 * C).astype(BF)
    wt = np.concatenate([wtap, wtap], axis=0)  # [128, 576]

    # sel bands, cnt rows permuted: cnt row u <-> out row ROWPERM[u].
    # group0 handles padded rows p=0..9 (covers out rows 0..7), group1
    # p=8..17 (out rows 8..15): sel[*, k*R + u] = 1 iff p-2 <= ROWPERM[u] <= p.
    selA = np.zeros((G, R), dtype=BF)
    selB = np.zeros((G, R), dtype=BF)
    for u in range(R):
        rr = ROWPERM[u]
        for p in range(rr, rr + 3):        # padded rows rr..rr+2
            if rr <= 7:
                selA[p, u] = 1
            else:
                selB[p - 8, u] = 1
    sel = np.empty((128, G * R), dtype=BF)
    sel[:64] = selA.reshape(1, G * R)
    sel[64:] = selB.reshape(1, G * R)

    sel2x = np.zeros((2, 128), dtype=BF)
    sel2x[0, :64] = 1
    sel2x[1, 64:] = 1
    biasv = np.tile(bias.astype(np.float32).reshape(-1, 1), (2, 1))  # [128,1]

    lrow = np.array(LROW)
    urow = np.array(UROW)

    in_maps = []
    for c in range(NCORES):
        r0 = c * RPC

        def slab(pad):
            s = np.empty((NT, 128, G * WP), dtype=BF)
            for t in range(NT):
                rows = pad[:, r0 + 16 * t: r0 + 16 * t + NPAD, :]  # [C,18,WP]
                s[t, :64] = rows[:, 0:10].reshape(C, G * WP)
                s[t, 64:] = rows[:, 8:18].reshape(C, G * WP)
            return s

        po = prev_output[0][:, r0:r0 + RPC, :].astype(BF)  # [C, 64, W]
        po = po.reshape(C, NT, R, W)
        pot = np.empty((NT, 128, 8 * W), dtype=BF)
        for t in range(NT):
            pot[t, :64] = po[:, t, lrow].reshape(C, 8 * W)
            pot[t, 64:] = po[:, t, urow].reshape(C, 8 * W)

        in_maps.append({
            "xin": slab(xpad), "pin": slab(ppad), "pout": pot,
            "wt": wt, "sel": sel, "sel2x": sel2x, "biasv": biasv,
        })
    return in_maps


def host_post(results):
    """Reassemble [NCORES] x [NT, 128, 8*W] bf16 -> [1, C, H, W] fp32."""
    out = np.empty((1, C, H, W), dtype=np.float32)
    lrow = np.array(LROW)
    urow = np.array(UROW)
    for c, res in enumerate(results):
        o = res["out"].reshape(NT, 2, C, 8, W).astype(np.float32)
        blk = np.empty((NT, C, R, W), dtype=np.float32)
        blk[:, :, lrow] = o[:, 0]
        blk[:, :, urow] = o[:, 1]
        out[0, :, c * RPC:(c + 1) * RPC, :] = \
            blk.transpose(1, 0, 2, 3).reshape(C, RPC, W)
    return out


def kernel(inp, prev_input, prev_output, weight, bias):
    if "nc" not in _cached:
        _cached["nc"] = build_nc(0)
    nc = _cached["nc"]
    in_maps = host_prep(inp, prev_input, prev_output, weight, bias)
    res = run_bass_kernel_spmd(nc, in_maps, core_ids=list(range(NCORES)))
    return host_post(res.results)


if __name__ == "__main__":
    rng = np.random.default_rng(0)
    inp = rng.standard_normal((1, C, H, W), dtype=np.float32)
    prev_input = inp + 0.05 * rng.standard_normal((1, C, H, W), dtype=np.float32)
    prev_output = rng.standard_normal((1, C, H, W), dtype=np.float32)
    weight = (0.05 * rng.standard_normal((C, C, 3, 3))).astype(np.float32)
    bias = rng.standard_normal(C).astype(np.float32)
    out = kernel(inp=inp, prev_input=prev_input, prev_output=prev_output,
                 weight=weight, bias=bias)
    print("out", out.shape, out.dtype, float(np.abs(out).mean()))
